# revision 5
# baseline (speedup 1.0000x reference)
"""Trainium2 Bass kernel for nn_DataReuploadingEncoder (4-qubit data
re-uploading circuit, B=1048576 samples, 8-core data parallel).

Algorithm: per qubit the encoding gate RZ(a/2)@RY(a) is decomposed via
RY(t) = S H RZ(t) H Sdag into two per-sample *diagonal* gates sandwiched by
fixed 1q gates; the fixed parts merge with the trainable Rot gates and the
CNOT ring into four constant 16x16 complex matrices.  Per layer:

    s = D1 s      # diag, phases Phi_j = sum_i sgn_ij * pi*scal_li*tanh(x_i)/2
    s = M_mid s   # kron^4(S H)                      (TensorE, 4 real matmuls)
    s = D2 s      # diag, phases Phi_j / 2
    s = M_l s     # fixed per-layer 16x16             (TensorE, 4 real matmuls)

Trig: only half-angle sin/cos are computed on ScalarE (args within the ACT
Sin spline domain [-pi,pi]); full-angle values come from double-angle
identities on VectorE.  Layer-0's D1 acts on the uniform state, so its
output is just (cos Phi | sin Phi) directly (1/4 scale folded into M_0).

Layout: state [128 partitions = 8 sample-groups x 16 state-index, 2 (re|im)
x 512 samples free].  Fixed gates are K=16 block-diagonal matmuls (8 blocks).
"""

import numpy as np

N_QUBITS = 4
N_LAYERS = 3
DIM = 16
G = 8          # sample groups per tile (partition packing)
FCOL = 512     # samples per group per tile (one PSUM bank of fp32)
F = G * FCOL   # samples per tile
N_CORES = 8

# ----------------------------------------------------------------------------
# host-side constant construction
# ----------------------------------------------------------------------------


def _rz(t):
    return np.diag([np.exp(-0.5j * t), np.exp(0.5j * t)]).astype(np.complex128)


def _ry(t):
    c, s = np.cos(t / 2), np.sin(t / 2)
    return np.array([[c, -s], [s, c]], dtype=np.complex128)


def _rot(phi, theta, omega):
    return _rz(omega) @ _ry(theta) @ _rz(phi)


def _kron4(mats):
    out = mats[0]
    for m in mats[1:]:
        out = np.kron(out, m)
    return out


def _cnot_mat(c, t):
    P = np.zeros((DIM, DIM), dtype=np.complex128)
    for j in range(DIM):
        bc = (j >> (3 - c)) & 1
        jj = j ^ (1 << (3 - t)) if bc else j
        P[jj, j] = 1.0
    return P


def _bit(j, i):
    return (j >> (3 - i)) & 1


def _build_constants(weights, scaling):
    """A-rows (phase matmul), 4 gate matrices, sign matrix; plus the map
    layer -> distinct-A index."""
    weights = np.asarray(weights, dtype=np.float64)
    scaling = np.asarray(scaling, dtype=np.float64)

    A = np.zeros((N_LAYERS, N_QUBITS, DIM))
    for l in range(N_LAYERS):
        for i in range(N_QUBITS):
            for j in range(DIM):
                sgn = 1.0 if _bit(j, i) else -1.0
                A[l, i, j] = sgn * np.pi * scaling[l, i] / 2.0

    # dedup identical scaling rows (harness uses all-ones -> u == 1)
    uniq = []
    lmap = []
    for l in range(N_LAYERS):
        for k, ku in enumerate(uniq):
            if np.array_equal(A[l], A[ku]):
                lmap.append(k)
                break
        else:
            uniq.append(l)
            lmap.append(len(uniq) - 1)
    A_u = A[uniq]  # [u, 4, 16]

    S = np.diag([1.0, 1.0j]).astype(np.complex128)
    H = np.array([[1, 1], [1, -1]], dtype=np.complex128) / np.sqrt(2.0)
    SH = S @ H
    HSd = H @ S.conj().T

    C = np.eye(DIM, dtype=np.complex128)
    for i in range(N_QUBITS):
        C = _cnot_mat(i, (i + 1) % N_QUBITS) @ C

    F_SH = _kron4([SH] * 4)
    F_HS = _kron4([HSd] * 4)
    R = [_kron4([_rot(*weights[l, i]) for i in range(N_QUBITS)])
         for l in range(N_LAYERS)]

    gates = [F_SH,
             0.25 * (F_HS @ C @ R[0]),
             F_HS @ C @ R[1],
             C @ R[2]]

    sign = np.zeros((DIM, N_QUBITS))
    for j in range(DIM):
        for w in range(N_QUBITS):
            sign[j, w] = 1.0 - 2.0 * _bit(j, w)

    return A_u, lmap, gates, sign


def _host_tensors(weights, scaling, dt_w=np.float32, dt_sq=np.float32):
    """Build the external-input constant tensors in device layout."""
    A_u, lmap, gates, sign = _build_constants(weights, scaling)
    u = A_u.shape[0]

    # phase matmul stationary: lhsT[(4g+i), (16g+j)] = A[l, i, j]
    phim = np.zeros((32, u, 128), dtype=np.float32)
    for g in range(G):
        for i in range(N_QUBITS):
            for l in range(u):
                phim[4 * g + i, l, 16 * g:16 * g + 16] = A_u[l, i]

    # gate stationaries: per gate {WrT, WiT, nWiT} block diag
    # lhsT[(16g+jin), (16g+jout)] = W[jout, jin]
    wmats = np.zeros((128, 12, 128), dtype=dt_w)
    for gi, M in enumerate(gates):
        Wr, Wi = np.real(M), np.imag(M)
        for kind, W in enumerate((Wr, Wi, -Wi)):
            blk = np.zeros((128, 128), dtype=np.float64)
            for g in range(G):
                blk[16 * g:16 * g + 16, 16 * g:16 * g + 16] = W.T
            wmats[:, gi * 3 + kind, :] = blk.astype(dt_w)

    # sign reduce stationary: lhsT[(16g+j), (4g+w)] = sign[j, w]
    signm = np.zeros((128, 32), dtype=dt_sq)
    for g in range(G):
        signm[16 * g:16 * g + 16, 4 * g:4 * g + 4] = sign.astype(dt_sq)

    return phim, wmats, signm, u, lmap


# ----------------------------------------------------------------------------
# bass kernel
# ----------------------------------------------------------------------------

_NC_CACHE = {}


def _build_nc(bs, u, lmap):
    import concourse.bass as bass
    import concourse.tile as tile
    from concourse import bacc, mybir
    from contextlib import ExitStack

    f32 = mybir.dt.float32
    ACT = mybir.ActivationFunctionType
    MULT = mybir.AluOpType.mult
    ADD = mybir.AluOpType.add
    SUB = mybir.AluOpType.subtract

    ntiles = bs // F
    assert bs % F == 0

    nc = bacc.Bacc("TRN2", target_bir_lowering=False, debug=False)
    x_ap = nc.dram_tensor("x", [bs, N_QUBITS], f32, kind="ExternalInput").ap()
    phim_ap = nc.dram_tensor("phimats", [32, u, 128], f32, kind="ExternalInput").ap()
    wm_ap = nc.dram_tensor("wmats", [128, 12, 128], f32, kind="ExternalInput").ap()
    sg_ap = nc.dram_tensor("signmat", [128, 32], f32, kind="ExternalInput").ap()
    out_ap = nc.dram_tensor("out", [bs, N_QUBITS], f32, kind="ExternalOutput").ap()

    # const bias for cos via Sin(pi/2 - x/2)
    halfpi = nc.alloc_sbuf_tensor("halfpi", [128, 1], f32)
    nc.gpsimd.memset(halfpi.ap(), float(np.pi / 2))
    nc.all_engine_barrier()

    with tile.TileContext(nc) as tc:
        with ExitStack() as ctx:
            consts = ctx.enter_context(tc.tile_pool(name="consts", bufs=1))
            inpool = ctx.enter_context(tc.tile_pool(name="inp", bufs=3))
            trig = ctx.enter_context(tc.tile_pool(name="trig", bufs=2))
            stp = ctx.enter_context(tc.tile_pool(name="state", bufs=2))
            tmpp = ctx.enter_context(tc.tile_pool(name="tmp", bufs=2))
            sqp = ctx.enter_context(tc.tile_pool(name="sq", bufs=2))
            phip = ctx.enter_context(tc.tile_pool(name="phip", bufs=1, space="PSUM"))
            gp = ctx.enter_context(tc.tile_pool(name="gp", bufs=1, space="PSUM"))
            op = ctx.enter_context(tc.tile_pool(name="op", bufs=2, space="PSUM"))

            # load constants once
            phim = consts.tile([32, u, 128], f32)
            nc.sync.dma_start(phim[:], phim_ap[:])
            wm = consts.tile([128, 12, 128], f32)
            nc.sync.dma_start(wm[:], wm_ap[:])
            sgm = consts.tile([128, 32], f32)
            nc.sync.dma_start(sgm[:], sg_ap[:])

            def cgate(P, gi, mre, mim, ctx_start=True):
                """P[:,0,:] = Wr re - Wi im ; P[:,1,:] = Wi re + Wr im."""
                nc.tensor.matmul(P[:, 0, :], wm[:, gi * 3 + 0, :], mre,
                                 start=True, stop=False)
                nc.tensor.matmul(P[:, 0, :], wm[:, gi * 3 + 2, :], mim,
                                 start=False, stop=True)
                nc.tensor.matmul(P[:, 1, :], wm[:, gi * 3 + 1, :], mre,
                                 start=True, stop=False)
                nc.tensor.matmul(P[:, 1, :], wm[:, gi * 3 + 0, :], mim,
                                 start=False, stop=True)

            def dapply(P, ct, st):
                """out = e^{i phi} * P  (P: [128,2,FCOL] psum, ct/st: [128,FCOL]).
                Returns (re, im) SBUF APs."""
                crep = ct[:].unsqueeze(1).to_broadcast((128, 2, FCOL))
                srep = st[:].unsqueeze(1).to_broadcast((128, 2, FCOL))
                t1 = tmpp.tile([128, 2, FCOL], f32, tag="t1")
                nc.vector.tensor_tensor(out=t1[:], in0=P[:], in1=crep, op=MULT)
                t2 = tmpp.tile([128, 2, FCOL], f32, tag="t2")
                nc.vector.tensor_tensor(out=t2[:], in0=P[:], in1=srep, op=MULT)
                sd = stp.tile([128, 2, FCOL], f32, tag="sd")
                nc.vector.tensor_tensor(out=sd[:, 0, :], in0=t1[:, 0, :],
                                        in1=t2[:, 1, :], op=SUB)
                nc.vector.tensor_tensor(out=sd[:, 1, :], in0=t1[:, 1, :],
                                        in1=t2[:, 0, :], op=ADD)
                return sd

            for k in range(ntiles):
                s0 = k * F
                xt = inpool.tile([32, FCOL], f32, tag="xt")
                for i in range(N_QUBITS):
                    xv = x_ap[s0:s0 + F, i].rearrange("(g c) -> g c", g=G)
                    nc.sync.dma_start(xt[i::N_QUBITS, :], xv)
                th = inpool.tile([32, FCOL], f32, tag="th")
                nc.scalar.activation(th[:], xt[:], ACT.Tanh)

                phi = phip.tile([128, u, FCOL], f32)
                for l in range(u):
                    nc.tensor.matmul(phi[:, l, :], phim[:, l, :], th[:],
                                     start=True, stop=True)

                c1s, s1s, c2s, s2s = [], [], [], []
                for l in range(u):
                    aphi = trig.tile([128, FCOL], f32, tag="aphi")
                    nc.scalar.activation(aphi[:], phi[:, l, :], ACT.Abs)
                    s2 = trig.tile([128, FCOL], f32, tag="s2")
                    nc.scalar.activation(s2[:], phi[:, l, :], ACT.Sin, scale=0.5)
                    c2 = trig.tile([128, FCOL], f32, tag="c2")
                    nc.scalar.activation(c2[:], aphi[:], ACT.Sin,
                                         bias=halfpi.ap(), scale=-0.5)
                    ssq = trig.tile([128, FCOL], f32, tag="ssq")
                    nc.scalar.activation(ssq[:], s2[:], ACT.Square)
                    c1 = trig.tile([128, FCOL], f32, tag="c1")
                    nc.vector.tensor_scalar(out=c1[:], in0=ssq[:], scalar1=-2.0,
                                            scalar2=1.0, op0=MULT, op1=ADD)
                    s1 = trig.tile([128, FCOL], f32, tag="s1")
                    nc.vector.scalar_tensor_tensor(out=s1[:], in0=s2[:],
                                                   scalar=2.0, in1=c2[:],
                                                   op0=MULT, op1=MULT)
                    c1s.append(c1); s1s.append(s1); c2s.append(c2); s2s.append(s2)

                mre, mim = c1s[lmap[0]][:], s1s[lmap[0]][:]
                P2 = None
                for l in range(N_LAYERS):
                    li = lmap[l]
                    if l > 0:
                        sd1 = dapply(P2, c1s[li], s1s[li])
                        mre, mim = sd1[:, 0, :], sd1[:, 1, :]
                    P1 = gp.tile([128, 2, FCOL], f32, tag="P1")
                    cgate(P1, 0, mre, mim)
                    sd2 = dapply(P1, c2s[li], s2s[li])
                    P2 = gp.tile([128, 2, FCOL], f32, tag="P2")
                    cgate(P2, 1 + l, sd2[:, 0, :], sd2[:, 1, :])

                sq = sqp.tile([128, 2, FCOL], f32)
                nc.scalar.activation(sq[:], P2[:], ACT.Square)
                outp = op.tile([32, FCOL], f32)
                nc.tensor.matmul(outp[:], sgm[:], sq[:, 0, :],
                                 start=True, stop=False)
                nc.tensor.matmul(outp[:], sgm[:], sq[:, 1, :],
                                 start=False, stop=True)
                ot = sqp.tile([32, FCOL], f32, tag="ot")
                nc.scalar.copy(ot[:], outp[:])
                for w in range(N_QUBITS):
                    ov = out_ap[s0:s0 + F, w].rearrange("(g c) -> g c", g=G)
                    nc.sync.dma_start(ov, ot[w::N_QUBITS, :])

    nc.compile()
    return nc


def _get_nc(bs, u, lmap):
    key = (bs, u, tuple(lmap))
    if key not in _NC_CACHE:
        _NC_CACHE[key] = _build_nc(bs, u, lmap)
    return _NC_CACHE[key]


def kernel(x, weights, scaling):
    from concourse.bass_utils import run_bass_kernel_spmd

    x = np.ascontiguousarray(np.asarray(x, dtype=np.float32))
    B = x.shape[0]
    phim, wmats, signm, u, lmap = _host_tensors(weights, scaling)

    chunk = N_CORES * F
    Bp = ((B + chunk - 1) // chunk) * chunk
    if Bp != B:
        xp = np.zeros((Bp, x.shape[1]), dtype=np.float32)
        xp[:B] = x
        x = xp
    bs = Bp // N_CORES

    nc = _get_nc(bs, u, lmap)
    xs = x.reshape(N_CORES, bs, x.shape[1])
    in_maps = [{"x": np.ascontiguousarray(xs[i]), "phimats": phim,
                "wmats": wmats, "signmat": signm} for i in range(N_CORES)]
    res = run_bass_kernel_spmd(nc, in_maps, core_ids=list(range(N_CORES)))
    out = np.concatenate([r["out"] for r in res.results], axis=0)
    return out[:B]


# revision 8
# speedup vs baseline: 2.6877x; 2.6877x over previous
"""Trainium2 Bass kernel for nn_DataReuploadingEncoder (4-qubit data
re-uploading circuit, B=1048576 samples, 8-core data parallel).

Algorithm: per qubit the encoding gate RZ(a/2)@RY(a) is decomposed via
RY(t) = S H RZ(t) H Sdag into two per-sample *diagonal* gates sandwiched by
fixed 1q gates; the fixed parts merge with the trainable Rot gates and the
CNOT ring into four constant 16x16 complex matrices.  Per layer:

    s = D1 s      # diag, phases Phi_j = sum_i sgn_ij * pi*scal_li*tanh(x_i)/2
    s = M_mid s   # kron^4(S H)                      (TensorE, 4 real matmuls)
    s = D2 s      # diag, phases Phi_j / 2
    s = M_l s     # fixed per-layer 16x16             (TensorE, 4 real matmuls)

Trig: only half-angle sin/cos are computed on ScalarE (args within the ACT
Sin spline domain [-pi,pi]); full-angle values come from double-angle
identities on VectorE.  Layer-0's D1 acts on the uniform state, so its
output is just (cos Phi | sin Phi) directly (1/4 scale folded into M_0).

Layout: state [128 partitions = 8 sample-groups x 16 state-index, 2 (re|im)
x 512 samples free].  Fixed gates are K=16 block-diagonal matmuls (8 blocks).

I/O: x and out are 4-wide interleaved in DRAM, so straight strided DMA would
need 4-byte descriptors (measured 16 us per tile-DMA).  Instead both sides
go through contiguous DMAs + TensorE transposes.  Sample mapping per core:
  s(p, t, k, g) = 1024*p + 32*t + 8*k + g
  (p in 0..127 partitions-of-flat-load, t tile, k 0..3, g group)
state column of group g within tile t:  c = 128*k + p.
"""

import numpy as np

N_QUBITS = 4
N_LAYERS = 3
DIM = 16
G = 8          # sample groups per tile (partition packing)
FCOL = 512     # samples per group per tile (one PSUM bank of fp32)
F = G * FCOL   # samples per tile
N_CORES = 8

# ----------------------------------------------------------------------------
# host-side constant construction
# ----------------------------------------------------------------------------


def _rz(t):
    return np.diag([np.exp(-0.5j * t), np.exp(0.5j * t)]).astype(np.complex128)


def _ry(t):
    c, s = np.cos(t / 2), np.sin(t / 2)
    return np.array([[c, -s], [s, c]], dtype=np.complex128)


def _rot(phi, theta, omega):
    return _rz(omega) @ _ry(theta) @ _rz(phi)


def _kron4(mats):
    out = mats[0]
    for m in mats[1:]:
        out = np.kron(out, m)
    return out


def _cnot_mat(c, t):
    P = np.zeros((DIM, DIM), dtype=np.complex128)
    for j in range(DIM):
        bc = (j >> (3 - c)) & 1
        jj = j ^ (1 << (3 - t)) if bc else j
        P[jj, j] = 1.0
    return P


def _bit(j, i):
    return (j >> (3 - i)) & 1


def _build_constants(weights, scaling):
    weights = np.asarray(weights, dtype=np.float64)
    scaling = np.asarray(scaling, dtype=np.float64)

    A = np.zeros((N_LAYERS, N_QUBITS, DIM))
    for l in range(N_LAYERS):
        for i in range(N_QUBITS):
            for j in range(DIM):
                sgn = 1.0 if _bit(j, i) else -1.0
                A[l, i, j] = sgn * np.pi * scaling[l, i] / 2.0

    # dedup identical scaling rows (harness uses all-ones -> u == 1)
    uniq = []
    lmap = []
    for l in range(N_LAYERS):
        for k, ku in enumerate(uniq):
            if np.array_equal(A[l], A[ku]):
                lmap.append(k)
                break
        else:
            uniq.append(l)
            lmap.append(len(uniq) - 1)
    A_u = A[uniq]  # [u, 4, 16]

    S = np.diag([1.0, 1.0j]).astype(np.complex128)
    H = np.array([[1, 1], [1, -1]], dtype=np.complex128) / np.sqrt(2.0)
    SH = S @ H
    HSd = H @ S.conj().T

    C = np.eye(DIM, dtype=np.complex128)
    for i in range(N_QUBITS):
        C = _cnot_mat(i, (i + 1) % N_QUBITS) @ C

    F_SH = _kron4([SH] * 4)
    F_HS = _kron4([HSd] * 4)
    R = [_kron4([_rot(*weights[l, i]) for i in range(N_QUBITS)])
         for l in range(N_LAYERS)]

    gates = [F_SH,
             0.25 * (F_HS @ C @ R[0]),
             F_HS @ C @ R[1],
             C @ R[2]]

    sign = np.zeros((DIM, N_QUBITS))
    for j in range(DIM):
        for w in range(N_QUBITS):
            sign[j, w] = 1.0 - 2.0 * _bit(j, w)

    return A_u, lmap, gates, sign


def _host_tensors(weights, scaling, dt_state=np.float32):
    A_u, lmap, gates, sign = _build_constants(weights, scaling)
    u = A_u.shape[0]

    # phase matmul stationaries: one full-K [128,128] per k-chunk, zero
    # outside rows [32k, 32k+32) -> no tile_position needed.
    # lhsT[k][(32k + 4g + i), l, (16g + j)] = A[l, i, j]
    phim = np.zeros((4, 128, u, 128), dtype=np.float32)
    for k in range(4):
        for g in range(G):
            for i in range(N_QUBITS):
                for l in range(u):
                    phim[k, 32 * k + 4 * g + i, l, 16 * g:16 * g + 16] = A_u[l, i]

    # gate stationaries: per gate {WrT, WiT, nWiT} block diag
    wmats = np.zeros((128, 12, 128), dtype=dt_state)
    for gi, M in enumerate(gates):
        Wr, Wi = np.real(M), np.imag(M)
        for kind, W in enumerate((Wr, Wi, -Wi)):
            blk = np.zeros((128, 128), dtype=np.float64)
            for g in range(G):
                blk[16 * g:16 * g + 16, 16 * g:16 * g + 16] = W.T
            wmats[:, gi * 3 + kind, :] = blk.astype(dt_state)

    # sign reduce stationary: lhsT[(16g+j), (4g+w)] = sign[j, w]
    signm = np.zeros((128, 32), dtype=dt_state)
    for g in range(G):
        signm[16 * g:16 * g + 16, 4 * g:4 * g + 4] = sign.astype(dt_state)

    # transpose identities
    id128 = np.eye(128, dtype=np.float32)
    id32 = np.eye(32, dtype=np.float32)

    return {"phimats": phim, "wmats": wmats, "signmat": signm,
            "id128": id128, "id32": id32}, u, lmap


# ----------------------------------------------------------------------------
# bass kernel
# ----------------------------------------------------------------------------

_NC_CACHE = {}

DT_STATE = "float32"   # knob: float32 / float16 / bfloat16


def _build_nc(bs, u, lmap, dt_state_name=None):
    import concourse.tile as tile
    from concourse import bacc, mybir
    from contextlib import ExitStack

    f32 = mybir.dt.float32
    dt_st = getattr(mybir.dt, dt_state_name or DT_STATE)
    ACT = mybir.ActivationFunctionType
    MULT = mybir.AluOpType.mult
    ADD = mybir.AluOpType.add
    SUB = mybir.AluOpType.subtract

    ntiles = bs // F
    assert bs % F == 0
    nflat = bs * N_QUBITS // 128          # elements per partition in flat load

    nc = bacc.Bacc("TRN2", target_bir_lowering=False, debug=False)
    x_ap = nc.dram_tensor("x", [bs, N_QUBITS], f32, kind="ExternalInput").ap()
    phim_ap = nc.dram_tensor("phimats", [4, 128, u, 128], f32,
                             kind="ExternalInput").ap()
    wm_ap = nc.dram_tensor("wmats", [128, 12, 128], dt_st,
                           kind="ExternalInput").ap()
    sg_ap = nc.dram_tensor("signmat", [128, 32], dt_st,
                           kind="ExternalInput").ap()
    id128_ap = nc.dram_tensor("id128", [128, 128], f32,
                              kind="ExternalInput").ap()
    id32_ap = nc.dram_tensor("id32", [32, 32], f32, kind="ExternalInput").ap()
    out_ap = nc.dram_tensor("out", [bs, N_QUBITS], f32,
                            kind="ExternalOutput").ap()

    halfpi = nc.alloc_sbuf_tensor("halfpi", [128, 1], f32)
    nc.gpsimd.memset(halfpi.ap(), float(np.pi / 2))
    nc.all_engine_barrier()

    with tile.TileContext(nc) as tc:
        with ExitStack() as ctx:
            consts = ctx.enter_context(tc.tile_pool(name="consts", bufs=1))
            bigp = ctx.enter_context(tc.tile_pool(name="big", bufs=1))
            trig = ctx.enter_context(tc.tile_pool(name="trig", bufs=2))
            stp = ctx.enter_context(tc.tile_pool(name="state", bufs=2))
            tmpp = ctx.enter_context(tc.tile_pool(name="tmp", bufs=2))
            sqp = ctx.enter_context(tc.tile_pool(name="sq", bufs=2))
            tpp = ctx.enter_context(tc.tile_pool(name="tpp", bufs=1, space="PSUM"))
            phip = ctx.enter_context(tc.tile_pool(name="phip", bufs=1, space="PSUM"))
            gp = ctx.enter_context(tc.tile_pool(name="gp", bufs=1, space="PSUM"))
            op = ctx.enter_context(tc.tile_pool(name="op", bufs=1, space="PSUM"))
            tap = ctx.enter_context(tc.tile_pool(name="tap", bufs=1, space="PSUM"))

            # constants
            phim = consts.tile([128, 4, u, 128], f32)
            for k in range(4):
                nc.sync.dma_start(phim[:, k, :, :], phim_ap[k, :, :, :])
            wm = consts.tile([128, 12, 128], dt_st)
            nc.sync.dma_start(wm[:], wm_ap[:])
            sgm = consts.tile([128, 32], dt_st)
            nc.sync.dma_start(sgm[:], sg_ap[:])
            id128 = consts.tile([128, 128], f32)
            nc.sync.dma_start(id128[:], id128_ap[:])
            id32 = consts.tile([32, 32], f32)
            nc.sync.dma_start(id32[:], id32_ap[:])

            # ---- phase 0: contiguous load, transpose, tanh  --------------
            fl = bigp.tile([128, nflat], f32)
            xflat = x_ap[:].rearrange("(p s) i -> p (s i)", p=128)
            nc.sync.dma_start(fl[:], xflat)
            th_all = bigp.tile([128, ntiles, 128], f32)
            for t in range(ntiles):
                tp = tpp.tile([128, 128], f32)
                nc.tensor.transpose(tp[:], fl[:, 128 * t:128 * (t + 1)],
                                    id128[:])
                nc.scalar.activation(th_all[:, t, :], tp[:], ACT.Tanh)

            od_all = bigp.tile([128, ntiles, 128], f32)

            def cgate(P, gi, mre, mim):
                nc.tensor.matmul(P[:, 0, :], wm[:, gi * 3 + 0, :], mre,
                                 start=True, stop=False)
                nc.tensor.matmul(P[:, 0, :], wm[:, gi * 3 + 2, :], mim,
                                 start=False, stop=True)
                nc.tensor.matmul(P[:, 1, :], wm[:, gi * 3 + 1, :], mre,
                                 start=True, stop=False)
                nc.tensor.matmul(P[:, 1, :], wm[:, gi * 3 + 0, :], mim,
                                 start=False, stop=True)

            def dapply(P, ct, st):
                """(re', im') = e^{i phi} * P, phi given by (ct, st)."""
                crep = ct[:].unsqueeze(1).to_broadcast((128, 2, FCOL))
                srep = st[:].unsqueeze(1).to_broadcast((128, 2, FCOL))
                t1 = tmpp.tile([128, 2, FCOL], dt_st, tag="t1")
                nc.vector.tensor_tensor(out=t1[:], in0=P[:], in1=crep, op=MULT)
                t2 = tmpp.tile([128, 2, FCOL], dt_st, tag="t2")
                nc.vector.tensor_tensor(out=t2[:], in0=P[:], in1=srep, op=MULT)
                sd = stp.tile([128, 2, FCOL], dt_st, tag="sd")
                nc.vector.tensor_tensor(out=sd[:, 0, :], in0=t1[:, 0, :],
                                        in1=t2[:, 1, :], op=SUB)
                nc.vector.tensor_tensor(out=sd[:, 1, :], in0=t1[:, 1, :],
                                        in1=t2[:, 0, :], op=ADD)
                return sd

            # ---- main loop ----------------------------------------------
            for t in range(ntiles):
                phi = phip.tile([128, u, FCOL], f32)
                for l in range(u):
                    for k in range(4):
                        nc.tensor.matmul(
                            phi[:, l, 128 * k:128 * (k + 1)],
                            phim[:, k, l, :],
                            th_all[:, t, :],
                            start=True, stop=True)

                c1s, s1s, c2s, s2s = [], [], [], []
                for l in range(u):
                    aphi = trig.tile([128, FCOL], f32, tag="aphi")
                    nc.scalar.activation(aphi[:], phi[:, l, :], ACT.Abs)
                    s2 = trig.tile([128, FCOL], dt_st, tag="s2")
                    nc.scalar.activation(s2[:], phi[:, l, :], ACT.Sin, scale=0.5)
                    c2 = trig.tile([128, FCOL], dt_st, tag="c2")
                    nc.scalar.activation(c2[:], aphi[:], ACT.Sin,
                                         bias=halfpi.ap(), scale=-0.5)
                    ssq = trig.tile([128, FCOL], dt_st, tag="ssq")
                    nc.scalar.activation(ssq[:], s2[:], ACT.Square)
                    c1 = trig.tile([128, FCOL], dt_st, tag="c1")
                    nc.vector.tensor_scalar(out=c1[:], in0=ssq[:], scalar1=-2.0,
                                            scalar2=1.0, op0=MULT, op1=ADD)
                    s1 = trig.tile([128, FCOL], dt_st, tag="s1")
                    nc.vector.scalar_tensor_tensor(out=s1[:], in0=s2[:],
                                                   scalar=2.0, in1=c2[:],
                                                   op0=MULT, op1=MULT)
                    c1s.append(c1); s1s.append(s1)
                    c2s.append(c2); s2s.append(s2)

                mre, mim = c1s[lmap[0]][:], s1s[lmap[0]][:]
                P2 = None
                for l in range(N_LAYERS):
                    li = lmap[l]
                    if l > 0:
                        sd1 = dapply(P2, c1s[li], s1s[li])
                        mre, mim = sd1[:, 0, :], sd1[:, 1, :]
                    P1 = gp.tile([128, 2, FCOL], f32, tag="P1")
                    cgate(P1, 0, mre, mim)
                    sd2 = dapply(P1, c2s[li], s2s[li])
                    P2 = gp.tile([128, 2, FCOL], f32, tag="P2")
                    cgate(P2, 1 + l, sd2[:, 0, :], sd2[:, 1, :])

                sq = sqp.tile([128, 2, FCOL], dt_st, tag="sqt")
                nc.scalar.activation(sq[:], P2[:], ACT.Square)
                outp = op.tile([32, FCOL], f32)
                nc.tensor.matmul(outp[:], sgm[:], sq[:, 0, :],
                                 start=True, stop=False)
                nc.tensor.matmul(outp[:], sgm[:], sq[:, 1, :],
                                 start=False, stop=True)
                ot = sqp.tile([32, FCOL], f32, tag="ot")
                nc.scalar.copy(ot[:], outp[:])
                ta = tap.tile([128, 4, 32], f32)
                for k in range(4):
                    nc.tensor.transpose(ta[:, k, :],
                                        ot[:, 128 * k:128 * (k + 1)], id32[:])
                nc.scalar.copy(od_all[:, t, :],
                               ta[:].rearrange("p a b -> p (a b)"))

            # ---- final store --------------------------------------------
            oflat = out_ap[:].rearrange("(p s) w -> p (s w)", p=128)
            nc.sync.dma_start(oflat, od_all[:].rearrange("p a b -> p (a b)"))

    nc.compile()
    return nc


def _get_nc(bs, u, lmap, dt_state_name=None):
    key = (bs, u, tuple(lmap), dt_state_name or DT_STATE)
    if key not in _NC_CACHE:
        _NC_CACHE[key] = _build_nc(bs, u, lmap, dt_state_name)
    return _NC_CACHE[key]


def _np_dt(name):
    import ml_dtypes
    return {"float32": np.float32, "float16": np.float16,
            "bfloat16": ml_dtypes.bfloat16}[name]


def kernel(x, weights, scaling):
    from concourse.bass_utils import run_bass_kernel_spmd

    x = np.ascontiguousarray(np.asarray(x, dtype=np.float32))
    B = x.shape[0]
    consts, u, lmap = _host_tensors(weights, scaling,
                                    dt_state=_np_dt(DT_STATE))

    chunk = N_CORES * F
    Bp = ((B + chunk - 1) // chunk) * chunk
    if Bp != B:
        xp = np.zeros((Bp, x.shape[1]), dtype=np.float32)
        xp[:B] = x
        x = xp
    bs = Bp // N_CORES

    nc = _get_nc(bs, u, lmap)
    xs = x.reshape(N_CORES, bs, x.shape[1])
    in_maps = [dict(consts, x=np.ascontiguousarray(xs[i]))
               for i in range(N_CORES)]
    res = run_bass_kernel_spmd(nc, in_maps, core_ids=list(range(N_CORES)))
    out = np.concatenate([r["out"] for r in res.results], axis=0)
    return out[:B]


# revision 9
# speedup vs baseline: 3.7401x; 1.3916x over previous
"""Trainium2 Bass kernel for nn_DataReuploadingEncoder (4-qubit data
re-uploading circuit, B=1048576 samples, 8-core data parallel).

Algorithm: per qubit the encoding gate RZ(a/2)@RY(a) is decomposed via
RY(t) = S H RZ(t) H Sdag into two per-sample *diagonal* gates sandwiched by
fixed 1q gates; the fixed parts merge with the trainable Rot gates and the
CNOT ring into four constant 16x16 complex matrices.  Per layer:

    s = D1 s      # diag, phases Phi_j = sum_i sgn_ij * pi*scal_li*tanh(x_i)/2
    s = M_mid s   # kron^4(S H)                      (TensorE, 4 real matmuls)
    s = D2 s      # diag, phases Phi_j / 2
    s = M_l s     # fixed per-layer 16x16             (TensorE, 4 real matmuls)

Trig: only half-angle sin/cos are computed on ScalarE (args within the ACT
Sin spline domain [-pi,pi]); full-angle values come from double-angle
identities on VectorE.  Layer-0's D1 acts on the uniform state, so its
output is just (cos Phi | sin Phi) directly (1/4 scale folded into M_0).

Layout: state [128 partitions = 8 sample-groups x 16 state-index, 2 (re|im)
x 512 samples free].  Fixed gates are K=16 block-diagonal matmuls (8 blocks).

I/O: x and out are 4-wide interleaved in DRAM, so straight strided DMA would
need 4-byte descriptors (measured 16 us per tile-DMA).  Instead both sides
go through contiguous DMAs + TensorE transposes.  Sample mapping per core:
  s(p, t, k, g) = 1024*p + 32*t + 8*k + g
  (p in 0..127 partitions-of-flat-load, t tile, k 0..3, g group)
state column of group g within tile t:  c = 128*k + p.
"""

import numpy as np

N_QUBITS = 4
N_LAYERS = 3
DIM = 16
G = 8          # sample groups per tile (partition packing)
FCOL = 512     # samples per group per tile (one PSUM bank of fp32)
F = G * FCOL   # samples per tile
N_CORES = 8

# ----------------------------------------------------------------------------
# host-side constant construction
# ----------------------------------------------------------------------------


def _rz(t):
    return np.diag([np.exp(-0.5j * t), np.exp(0.5j * t)]).astype(np.complex128)


def _ry(t):
    c, s = np.cos(t / 2), np.sin(t / 2)
    return np.array([[c, -s], [s, c]], dtype=np.complex128)


def _rot(phi, theta, omega):
    return _rz(omega) @ _ry(theta) @ _rz(phi)


def _kron4(mats):
    out = mats[0]
    for m in mats[1:]:
        out = np.kron(out, m)
    return out


def _cnot_mat(c, t):
    P = np.zeros((DIM, DIM), dtype=np.complex128)
    for j in range(DIM):
        bc = (j >> (3 - c)) & 1
        jj = j ^ (1 << (3 - t)) if bc else j
        P[jj, j] = 1.0
    return P


def _bit(j, i):
    return (j >> (3 - i)) & 1


def _build_constants(weights, scaling):
    weights = np.asarray(weights, dtype=np.float64)
    scaling = np.asarray(scaling, dtype=np.float64)

    A = np.zeros((N_LAYERS, N_QUBITS, DIM))
    for l in range(N_LAYERS):
        for i in range(N_QUBITS):
            for j in range(DIM):
                sgn = 1.0 if _bit(j, i) else -1.0
                A[l, i, j] = sgn * np.pi * scaling[l, i] / 2.0

    # dedup identical scaling rows (harness uses all-ones -> u == 1)
    uniq = []
    lmap = []
    for l in range(N_LAYERS):
        for k, ku in enumerate(uniq):
            if np.array_equal(A[l], A[ku]):
                lmap.append(k)
                break
        else:
            uniq.append(l)
            lmap.append(len(uniq) - 1)
    A_u = A[uniq]  # [u, 4, 16]

    S = np.diag([1.0, 1.0j]).astype(np.complex128)
    H = np.array([[1, 1], [1, -1]], dtype=np.complex128) / np.sqrt(2.0)
    SH = S @ H
    HSd = H @ S.conj().T

    C = np.eye(DIM, dtype=np.complex128)
    for i in range(N_QUBITS):
        C = _cnot_mat(i, (i + 1) % N_QUBITS) @ C

    F_SH = _kron4([SH] * 4)
    F_HS = _kron4([HSd] * 4)
    R = [_kron4([_rot(*weights[l, i]) for i in range(N_QUBITS)])
         for l in range(N_LAYERS)]

    gates = [F_SH,
             0.25 * (F_HS @ C @ R[0]),
             F_HS @ C @ R[1],
             C @ R[2]]

    sign = np.zeros((DIM, N_QUBITS))
    for j in range(DIM):
        for w in range(N_QUBITS):
            sign[j, w] = 1.0 - 2.0 * _bit(j, w)

    return A_u, lmap, gates, sign


def _host_tensors(weights, scaling, dt_state=np.float32):
    A_u, lmap, gates, sign = _build_constants(weights, scaling)
    u = A_u.shape[0]

    # phase matmul stationaries: one full-K [128,128] per k-chunk, zero
    # outside rows [32k, 32k+32) -> no tile_position needed.
    # lhsT[k][(32k + 4g + i), l, (16g + j)] = A[l, i, j]
    phim = np.zeros((4, 128, u, 128), dtype=np.float32)
    for k in range(4):
        for g in range(G):
            for i in range(N_QUBITS):
                for l in range(u):
                    phim[k, 32 * k + 4 * g + i, l, 16 * g:16 * g + 16] = A_u[l, i]

    # gate stationaries: per gate {WrT, WiT, nWiT} block diag
    wmats = np.zeros((128, 12, 128), dtype=dt_state)
    for gi, M in enumerate(gates):
        Wr, Wi = np.real(M), np.imag(M)
        for kind, W in enumerate((Wr, Wi, -Wi)):
            blk = np.zeros((128, 128), dtype=np.float64)
            for g in range(G):
                blk[16 * g:16 * g + 16, 16 * g:16 * g + 16] = W.T
            wmats[:, gi * 3 + kind, :] = blk.astype(dt_state)

    # sign reduce stationary: lhsT[(16g+j), (4g+w)] = sign[j, w]
    signm = np.zeros((128, 32), dtype=dt_state)
    for g in range(G):
        signm[16 * g:16 * g + 16, 4 * g:4 * g + 4] = sign.astype(dt_state)

    # transpose identities
    id128 = np.eye(128, dtype=np.float32)
    id32 = np.eye(32, dtype=np.float32)

    return {"phimats": phim, "wmats": wmats, "signmat": signm,
            "id128": id128, "id32": id32}, u, lmap


# ----------------------------------------------------------------------------
# bass kernel
# ----------------------------------------------------------------------------

_NC_CACHE = {}

DT_STATE = "float32r"  # knob: float32 / float32r / float16 / bfloat16
ADDSUB_ENGINE = "gpsimd"  # knob: "vector" | "gpsimd"


def _build_nc(bs, u, lmap, dt_state_name=None):
    import concourse.tile as tile
    from concourse import bacc, mybir
    from contextlib import ExitStack

    f32 = mybir.dt.float32
    dt_st = getattr(mybir.dt, dt_state_name or DT_STATE)
    ACT = mybir.ActivationFunctionType
    MULT = mybir.AluOpType.mult
    ADD = mybir.AluOpType.add
    SUB = mybir.AluOpType.subtract

    ntiles = bs // F
    assert bs % F == 0
    nflat = bs * N_QUBITS // 128          # elements per partition in flat load

    nc = bacc.Bacc("TRN2", target_bir_lowering=False, debug=False)
    x_ap = nc.dram_tensor("x", [bs, N_QUBITS], f32, kind="ExternalInput").ap()
    phim_ap = nc.dram_tensor("phimats", [4, 128, u, 128], f32,
                             kind="ExternalInput").ap()
    wm_ap = nc.dram_tensor("wmats", [128, 12, 128], dt_st,
                           kind="ExternalInput").ap()
    sg_ap = nc.dram_tensor("signmat", [128, 32], dt_st,
                           kind="ExternalInput").ap()
    id128_ap = nc.dram_tensor("id128", [128, 128], f32,
                              kind="ExternalInput").ap()
    id32_ap = nc.dram_tensor("id32", [32, 32], f32, kind="ExternalInput").ap()
    out_ap = nc.dram_tensor("out", [bs, N_QUBITS], f32,
                            kind="ExternalOutput").ap()

    halfpi = nc.alloc_sbuf_tensor("halfpi", [128, 1], f32)
    nc.gpsimd.memset(halfpi.ap(), float(np.pi / 2))
    nc.all_engine_barrier()

    with tile.TileContext(nc) as tc:
        with ExitStack() as ctx:
            consts = ctx.enter_context(tc.tile_pool(name="consts", bufs=1))
            bigp = ctx.enter_context(tc.tile_pool(name="big", bufs=1))
            trig = ctx.enter_context(tc.tile_pool(name="trig", bufs=2))
            stp = ctx.enter_context(tc.tile_pool(name="state", bufs=2))
            tmpp = ctx.enter_context(tc.tile_pool(name="tmp", bufs=2))
            sqp = ctx.enter_context(tc.tile_pool(name="sq", bufs=2))
            tpp = ctx.enter_context(tc.tile_pool(name="tpp", bufs=1, space="PSUM"))
            phip = ctx.enter_context(tc.tile_pool(name="phip", bufs=1, space="PSUM"))
            gp = ctx.enter_context(tc.tile_pool(name="gp", bufs=1, space="PSUM"))
            op = ctx.enter_context(tc.tile_pool(name="op", bufs=1, space="PSUM"))
            tap = ctx.enter_context(tc.tile_pool(name="tap", bufs=1, space="PSUM"))

            # constants
            phim = consts.tile([128, 4, u, 128], f32)
            for k in range(4):
                nc.sync.dma_start(phim[:, k, :, :], phim_ap[k, :, :, :])
            wm = consts.tile([128, 12, 128], dt_st)
            nc.sync.dma_start(wm[:], wm_ap[:])
            sgm = consts.tile([128, 32], dt_st)
            nc.sync.dma_start(sgm[:], sg_ap[:])
            id128 = consts.tile([128, 128], f32)
            nc.sync.dma_start(id128[:], id128_ap[:])
            id32 = consts.tile([32, 32], f32)
            nc.sync.dma_start(id32[:], id32_ap[:])

            # ---- phase 0: contiguous load, transpose, tanh  --------------
            fl = bigp.tile([128, nflat], f32)
            xflat = x_ap[:].rearrange("(p s) i -> p (s i)", p=128)
            nc.sync.dma_start(fl[:], xflat)
            th_all = bigp.tile([128, ntiles, 128], f32)
            for t in range(ntiles):
                tp = tpp.tile([128, 128], f32)
                nc.tensor.transpose(tp[:], fl[:, 128 * t:128 * (t + 1)],
                                    id128[:])
                nc.scalar.activation(th_all[:, t, :], tp[:], ACT.Tanh)

            od_all = bigp.tile([128, ntiles, 128], f32)

            def cgate(P, gi, mre, mim):
                nc.tensor.matmul(P[:, 0, :], wm[:, gi * 3 + 0, :], mre,
                                 start=True, stop=False)
                nc.tensor.matmul(P[:, 0, :], wm[:, gi * 3 + 2, :], mim,
                                 start=False, stop=True)
                nc.tensor.matmul(P[:, 1, :], wm[:, gi * 3 + 1, :], mre,
                                 start=True, stop=False)
                nc.tensor.matmul(P[:, 1, :], wm[:, gi * 3 + 0, :], mim,
                                 start=False, stop=True)

            aseng = nc.gpsimd if ADDSUB_ENGINE == "gpsimd" else nc.vector

            def dapply(P, ct, st):
                """(re', im') = e^{i phi} * P, phi given by (ct, st)."""
                crep = ct[:].unsqueeze(1).to_broadcast((128, 2, FCOL))
                srep = st[:].unsqueeze(1).to_broadcast((128, 2, FCOL))
                t1 = tmpp.tile([128, 2, FCOL], f32, tag="t1")
                nc.vector.tensor_tensor(out=t1[:], in0=P[:], in1=crep, op=MULT)
                t2 = tmpp.tile([128, 2, FCOL], f32, tag="t2")
                nc.vector.tensor_tensor(out=t2[:], in0=P[:], in1=srep, op=MULT)
                sd = stp.tile([128, 2, FCOL], dt_st, tag="sd")
                aseng.tensor_tensor(out=sd[:, 0, :], in0=t1[:, 0, :],
                                    in1=t2[:, 1, :], op=SUB)
                aseng.tensor_tensor(out=sd[:, 1, :], in0=t1[:, 1, :],
                                    in1=t2[:, 0, :], op=ADD)
                return sd

            # ---- main loop ----------------------------------------------
            for t in range(ntiles):
                phi = phip.tile([128, u, FCOL], f32)
                for l in range(u):
                    for k in range(4):
                        nc.tensor.matmul(
                            phi[:, l, 128 * k:128 * (k + 1)],
                            phim[:, k, l, :],
                            th_all[:, t, :],
                            start=True, stop=True)

                c1s, s1s, c2s, s2s = [], [], [], []
                for l in range(u):
                    aphi = trig.tile([128, FCOL], f32, tag="aphi")
                    nc.scalar.activation(aphi[:], phi[:, l, :], ACT.Abs)
                    s2 = trig.tile([128, FCOL], dt_st, tag="s2")
                    nc.scalar.activation(s2[:], phi[:, l, :], ACT.Sin, scale=0.5)
                    c2 = trig.tile([128, FCOL], dt_st, tag="c2")
                    nc.scalar.activation(c2[:], aphi[:], ACT.Sin,
                                         bias=halfpi.ap(), scale=-0.5)
                    ssq = trig.tile([128, FCOL], dt_st, tag="ssq")
                    nc.scalar.activation(ssq[:], s2[:], ACT.Square)
                    c1 = trig.tile([128, FCOL], dt_st, tag="c1")
                    nc.vector.tensor_scalar(out=c1[:], in0=ssq[:], scalar1=-2.0,
                                            scalar2=1.0, op0=MULT, op1=ADD)
                    s1 = trig.tile([128, FCOL], dt_st, tag="s1")
                    nc.vector.scalar_tensor_tensor(out=s1[:], in0=s2[:],
                                                   scalar=2.0, in1=c2[:],
                                                   op0=MULT, op1=MULT)
                    c1s.append(c1); s1s.append(s1)
                    c2s.append(c2); s2s.append(s2)

                mre, mim = c1s[lmap[0]][:], s1s[lmap[0]][:]
                P2 = None
                for l in range(N_LAYERS):
                    li = lmap[l]
                    if l > 0:
                        sd1 = dapply(P2, c1s[li], s1s[li])
                        mre, mim = sd1[:, 0, :], sd1[:, 1, :]
                    P1 = gp.tile([128, 2, FCOL], f32, tag="P1")
                    cgate(P1, 0, mre, mim)
                    sd2 = dapply(P1, c2s[li], s2s[li])
                    P2 = gp.tile([128, 2, FCOL], f32, tag="P2")
                    cgate(P2, 1 + l, sd2[:, 0, :], sd2[:, 1, :])

                sq = sqp.tile([128, 2, FCOL], dt_st, tag="sqt")
                nc.scalar.activation(sq[:], P2[:], ACT.Square)
                outp = op.tile([32, FCOL], f32)
                nc.tensor.matmul(outp[:], sgm[:], sq[:, 0, :],
                                 start=True, stop=False)
                nc.tensor.matmul(outp[:], sgm[:], sq[:, 1, :],
                                 start=False, stop=True)
                ot = sqp.tile([32, FCOL], f32, tag="ot")
                nc.scalar.copy(ot[:], outp[:])
                ta = tap.tile([128, 4, 32], f32)
                for k in range(4):
                    nc.tensor.transpose(ta[:, k, :],
                                        ot[:, 128 * k:128 * (k + 1)], id32[:])
                nc.scalar.copy(od_all[:, t, :],
                               ta[:].rearrange("p a b -> p (a b)"))

            # ---- final store --------------------------------------------
            oflat = out_ap[:].rearrange("(p s) w -> p (s w)", p=128)
            nc.sync.dma_start(oflat, od_all[:].rearrange("p a b -> p (a b)"))

    nc.compile()
    return nc


def _get_nc(bs, u, lmap, dt_state_name=None):
    key = (bs, u, tuple(lmap), dt_state_name or DT_STATE)
    if key not in _NC_CACHE:
        _NC_CACHE[key] = _build_nc(bs, u, lmap, dt_state_name)
    return _NC_CACHE[key]


def _np_dt(name):
    import ml_dtypes
    return {"float32": np.float32, "float32r": np.float32,
            "float16": np.float16, "bfloat16": ml_dtypes.bfloat16}[name]


def kernel(x, weights, scaling):
    from concourse.bass_utils import run_bass_kernel_spmd

    x = np.ascontiguousarray(np.asarray(x, dtype=np.float32))
    B = x.shape[0]
    consts, u, lmap = _host_tensors(weights, scaling,
                                    dt_state=_np_dt(DT_STATE))

    chunk = N_CORES * F
    Bp = ((B + chunk - 1) // chunk) * chunk
    if Bp != B:
        xp = np.zeros((Bp, x.shape[1]), dtype=np.float32)
        xp[:B] = x
        x = xp
    bs = Bp // N_CORES

    nc = _get_nc(bs, u, lmap)
    xs = x.reshape(N_CORES, bs, x.shape[1])
    in_maps = [dict(consts, x=np.ascontiguousarray(xs[i]))
               for i in range(N_CORES)]
    res = run_bass_kernel_spmd(nc, in_maps, core_ids=list(range(N_CORES)))
    out = np.concatenate([r["out"] for r in res.results], axis=0)
    return out[:B]


# revision 11
# speedup vs baseline: 3.9019x; 1.0433x over previous
"""Trainium2 Bass kernel for nn_DataReuploadingEncoder (4-qubit data
re-uploading circuit, B=1048576 samples, 8-core data parallel).

Algorithm: per qubit the encoding gate RZ(a/2)@RY(a) is decomposed via
RY(t) = S H RZ(t) H Sdag into two per-sample *diagonal* gates sandwiched by
fixed 1q gates; the fixed parts merge with the trainable Rot gates and the
CNOT ring into four constant 16x16 complex matrices.  Per layer:

    s = D1 s      # diag, phases Phi_j = sum_i sgn_ij * pi*scal_li*tanh(x_i)/2
    s = M_mid s   # kron^4(S H)                      (TensorE, 4 real matmuls)
    s = D2 s      # diag, phases Phi_j / 2
    s = M_l s     # fixed per-layer 16x16             (TensorE, 4 real matmuls)

Trig: only half-angle sin/cos are computed on ScalarE (args within the ACT
Sin spline domain [-pi,pi]); full-angle values come from double-angle
identities on VectorE.  Layer-0's D1 acts on the uniform state, so its
output is just (cos Phi | sin Phi) directly (1/4 scale folded into M_0).

Layout: state [128 partitions = 8 sample-groups x 16 state-index, 2 (re|im)
x 512 samples free].  Fixed gates are K=16 block-diagonal matmuls (8 blocks).

I/O: x and out are 4-wide interleaved in DRAM, so straight strided DMA would
need 4-byte descriptors (measured 16 us per tile-DMA).  Instead both sides
go through contiguous DMAs + TensorE transposes.  Sample mapping per core:
  s(p, t, k, g) = 1024*p + 32*t + 8*k + g
  (p in 0..127 partitions-of-flat-load, t tile, k 0..3, g group)
state column of group g within tile t:  c = 128*k + p.
"""

import numpy as np

N_QUBITS = 4
N_LAYERS = 3
DIM = 16
G = 8          # sample groups per tile (partition packing)
FCOL = 512     # samples per group per tile (one PSUM bank of fp32)
F = G * FCOL   # samples per tile
N_CORES = 8

# ----------------------------------------------------------------------------
# host-side constant construction
# ----------------------------------------------------------------------------


def _rz(t):
    return np.diag([np.exp(-0.5j * t), np.exp(0.5j * t)]).astype(np.complex128)


def _ry(t):
    c, s = np.cos(t / 2), np.sin(t / 2)
    return np.array([[c, -s], [s, c]], dtype=np.complex128)


def _rot(phi, theta, omega):
    return _rz(omega) @ _ry(theta) @ _rz(phi)


def _kron4(mats):
    out = mats[0]
    for m in mats[1:]:
        out = np.kron(out, m)
    return out


def _cnot_mat(c, t):
    P = np.zeros((DIM, DIM), dtype=np.complex128)
    for j in range(DIM):
        bc = (j >> (3 - c)) & 1
        jj = j ^ (1 << (3 - t)) if bc else j
        P[jj, j] = 1.0
    return P


def _bit(j, i):
    return (j >> (3 - i)) & 1


def _build_constants(weights, scaling):
    weights = np.asarray(weights, dtype=np.float64)
    scaling = np.asarray(scaling, dtype=np.float64)

    A = np.zeros((N_LAYERS, N_QUBITS, DIM))
    for l in range(N_LAYERS):
        for i in range(N_QUBITS):
            for j in range(DIM):
                sgn = 1.0 if _bit(j, i) else -1.0
                A[l, i, j] = sgn * np.pi * scaling[l, i] / 2.0

    # dedup identical scaling rows (harness uses all-ones -> u == 1)
    uniq = []
    lmap = []
    for l in range(N_LAYERS):
        for k, ku in enumerate(uniq):
            if np.array_equal(A[l], A[ku]):
                lmap.append(k)
                break
        else:
            uniq.append(l)
            lmap.append(len(uniq) - 1)
    A_u = A[uniq]  # [u, 4, 16]

    S = np.diag([1.0, 1.0j]).astype(np.complex128)
    H = np.array([[1, 1], [1, -1]], dtype=np.complex128) / np.sqrt(2.0)
    SH = S @ H
    HSd = H @ S.conj().T

    C = np.eye(DIM, dtype=np.complex128)
    for i in range(N_QUBITS):
        C = _cnot_mat(i, (i + 1) % N_QUBITS) @ C

    F_SH = _kron4([SH] * 4)
    F_HS = _kron4([HSd] * 4)
    R = [_kron4([_rot(*weights[l, i]) for i in range(N_QUBITS)])
         for l in range(N_LAYERS)]

    gates = [F_SH,
             0.25 * (F_HS @ C @ R[0]),
             F_HS @ C @ R[1],
             C @ R[2]]

    sign = np.zeros((DIM, N_QUBITS))
    for j in range(DIM):
        for w in range(N_QUBITS):
            sign[j, w] = 1.0 - 2.0 * _bit(j, w)

    return A_u, lmap, gates, sign


def _host_tensors(weights, scaling, dt_state=np.float32):
    A_u, lmap, gates, sign = _build_constants(weights, scaling)
    u = A_u.shape[0]

    # phase matmul stationaries: one full-K [128,128] per k-chunk, zero
    # outside rows [32k, 32k+32) -> no tile_position needed.
    # lhsT[k][(32k + 4g + i), l, (16g + j)] = A[l, i, j]
    phim = np.zeros((4, 128, u, 128), dtype=np.float32)
    for k in range(4):
        for g in range(G):
            for i in range(N_QUBITS):
                for l in range(u):
                    phim[k, 32 * k + 4 * g + i, l, 16 * g:16 * g + 16] = A_u[l, i]

    # gate stationaries: per gate {WrT, WiT, nWiT} block diag
    wmats = np.zeros((128, 12, 128), dtype=dt_state)
    for gi, M in enumerate(gates):
        Wr, Wi = np.real(M), np.imag(M)
        for kind, W in enumerate((Wr, Wi, -Wi)):
            blk = np.zeros((128, 128), dtype=np.float64)
            for g in range(G):
                blk[16 * g:16 * g + 16, 16 * g:16 * g + 16] = W.T
            wmats[:, gi * 3 + kind, :] = blk.astype(dt_state)

    # sign reduce stationary: lhsT[(16g+j), (4g+w)] = sign[j, w]
    signm = np.zeros((128, 32), dtype=dt_state)
    for g in range(G):
        signm[16 * g:16 * g + 16, 4 * g:4 * g + 4] = sign.astype(dt_state)

    # transpose identities
    id128 = np.eye(128, dtype=np.float32)
    id32 = np.eye(32, dtype=np.float32)

    return {"phimats": phim, "wmats": wmats, "signmat": signm,
            "id128": id128, "id32": id32}, u, lmap


# ----------------------------------------------------------------------------
# bass kernel
# ----------------------------------------------------------------------------

_NC_CACHE = {}

DT_STATE = "float32r"  # knob: float32 / float32r / float16 / bfloat16
ADDSUB_ENGINE = "gpsimd"  # knob: "vector" | "gpsimd"


def _build_nc(bs, u, lmap, dt_state_name=None):
    import concourse.tile as tile
    from concourse import bacc, mybir
    from contextlib import ExitStack

    f32 = mybir.dt.float32
    dt_st = getattr(mybir.dt, dt_state_name or DT_STATE)
    ACT = mybir.ActivationFunctionType
    MULT = mybir.AluOpType.mult
    ADD = mybir.AluOpType.add
    SUB = mybir.AluOpType.subtract

    ntiles = bs // F
    assert bs % F == 0
    nflat = bs * N_QUBITS // 128          # elements per partition in flat load

    nc = bacc.Bacc("TRN2", target_bir_lowering=False, debug=False)
    x_ap = nc.dram_tensor("x", [bs, N_QUBITS], f32, kind="ExternalInput").ap()
    phim_ap = nc.dram_tensor("phimats", [4, 128, u, 128], f32,
                             kind="ExternalInput").ap()
    wm_ap = nc.dram_tensor("wmats", [128, 12, 128], dt_st,
                           kind="ExternalInput").ap()
    sg_ap = nc.dram_tensor("signmat", [128, 32], dt_st,
                           kind="ExternalInput").ap()
    id128_ap = nc.dram_tensor("id128", [128, 128], f32,
                              kind="ExternalInput").ap()
    id32_ap = nc.dram_tensor("id32", [32, 32], f32, kind="ExternalInput").ap()
    out_ap = nc.dram_tensor("out", [bs, N_QUBITS], f32,
                            kind="ExternalOutput").ap()

    halfpi = nc.alloc_sbuf_tensor("halfpi", [128, 1], f32)
    nc.gpsimd.memset(halfpi.ap(), float(np.pi / 2))
    nc.all_engine_barrier()

    with tile.TileContext(nc) as tc:
        with ExitStack() as ctx:
            consts = ctx.enter_context(tc.tile_pool(name="consts", bufs=1))
            bigp = ctx.enter_context(tc.tile_pool(name="big", bufs=1))
            trig = ctx.enter_context(tc.tile_pool(name="trig", bufs=2))
            stp = ctx.enter_context(tc.tile_pool(name="state", bufs=2))
            tmpp = ctx.enter_context(tc.tile_pool(name="tmp", bufs=2))
            sqp = ctx.enter_context(tc.tile_pool(name="sq", bufs=2))
            tpp = ctx.enter_context(tc.tile_pool(name="tpp", bufs=1, space="PSUM"))
            phip = ctx.enter_context(tc.tile_pool(name="phip", bufs=2, space="PSUM"))
            gp = ctx.enter_context(tc.tile_pool(name="gp", bufs=1, space="PSUM"))
            op = ctx.enter_context(tc.tile_pool(name="op", bufs=1, space="PSUM"))

            # constants
            phim = consts.tile([128, 4, u, 128], f32)
            for k in range(4):
                nc.sync.dma_start(phim[:, k, :, :], phim_ap[k, :, :, :])
            wm = consts.tile([128, 12, 128], dt_st)
            nc.sync.dma_start(wm[:], wm_ap[:])
            sgm = consts.tile([128, 32], dt_st)
            nc.sync.dma_start(sgm[:], sg_ap[:])
            id128 = consts.tile([128, 128], f32)
            nc.sync.dma_start(id128[:], id128_ap[:])
            id32 = consts.tile([32, 32], f32)
            nc.sync.dma_start(id32[:], id32_ap[:])

            # ---- phase 0: contiguous load, transpose, tanh  --------------
            fl = bigp.tile([128, nflat], f32)
            xflat = x_ap[:].rearrange("(p s) i -> p (s i)", p=128)
            nc.sync.dma_start(fl[:], xflat)
            th_all = bigp.tile([128, ntiles, 128], f32)
            for t in range(ntiles):
                tp = tpp.tile([128, 128], f32, tag="x")
                nc.tensor.transpose(tp[:], fl[:, 128 * t:128 * (t + 1)],
                                    id128[:])
                nc.scalar.activation(th_all[:, t, :], tp[:], ACT.Tanh)

            od_all = bigp.tile([128, ntiles, 128], f32)

            def cgate(P, gi, mre, mim):
                # LDW-minimizing order: Wr used twice back-to-back
                nc.tensor.matmul(P[:, 0, :], wm[:, gi * 3 + 0, :], mre,
                                 start=True, stop=False)
                nc.tensor.matmul(P[:, 1, :], wm[:, gi * 3 + 0, :], mim,
                                 start=True, stop=False)
                nc.tensor.matmul(P[:, 1, :], wm[:, gi * 3 + 1, :], mre,
                                 start=False, stop=True)
                nc.tensor.matmul(P[:, 0, :], wm[:, gi * 3 + 2, :], mim,
                                 start=False, stop=True)

            aseng = nc.gpsimd if ADDSUB_ENGINE == "gpsimd" else nc.vector

            def dapply(P, cst):
                """(re', im') = e^{i phi} * P, phi given by cs tile [128,2,F]
                (cs[:,0,:]=cos, cs[:,1,:]=sin)."""
                # tb[:, 0, c, :] = cos*P[c]; tb[:, 1, c, :] = sin*P[c]
                prep = P[:].unsqueeze(1).to_broadcast((128, 2, 2, FCOL))
                csrep = cst[:].unsqueeze(2).to_broadcast((128, 2, 2, FCOL))
                tb = tmpp.tile([128, 2, 2, FCOL], f32, tag="tb")
                nc.vector.tensor_tensor(out=tb[:], in0=prep, in1=csrep, op=MULT)
                sd = stp.tile([128, 2, FCOL], dt_st, tag="sd")
                aseng.tensor_tensor(out=sd[:, 0, :], in0=tb[:, 0, 0, :],
                                    in1=tb[:, 1, 1, :], op=SUB)
                aseng.tensor_tensor(out=sd[:, 1, :], in0=tb[:, 0, 1, :],
                                    in1=tb[:, 1, 0, :], op=ADD)
                return sd

            # ---- main loop ----------------------------------------------
            for t in range(ntiles):
                phi = phip.tile([128, u, FCOL], f32)
                for l in range(u):
                    for k in range(4):
                        nc.tensor.matmul(
                            phi[:, l, 128 * k:128 * (k + 1)],
                            phim[:, k, l, :],
                            th_all[:, t, :],
                            start=True, stop=True)

                cs1s, cs2s = [], []
                for l in range(u):
                    aphi = trig.tile([128, FCOL], f32, tag="aphi")
                    nc.scalar.activation(aphi[:], phi[:, l, :], ACT.Abs)
                    cs2 = trig.tile([128, 2, FCOL], dt_st, tag="cs2")
                    nc.scalar.activation(cs2[:, 1, :], phi[:, l, :], ACT.Sin,
                                         scale=0.5)
                    nc.scalar.activation(cs2[:, 0, :], aphi[:], ACT.Sin,
                                         bias=halfpi.ap(), scale=-0.5)
                    ssq = trig.tile([128, FCOL], dt_st, tag="ssq")
                    nc.scalar.activation(ssq[:], cs2[:, 1, :], ACT.Square)
                    cs1 = trig.tile([128, 2, FCOL], dt_st, tag="cs1")
                    nc.vector.tensor_scalar(out=cs1[:, 0, :], in0=ssq[:],
                                            scalar1=-2.0, scalar2=1.0,
                                            op0=MULT, op1=ADD)
                    nc.vector.scalar_tensor_tensor(out=cs1[:, 1, :],
                                                   in0=cs2[:, 1, :], scalar=2.0,
                                                   in1=cs2[:, 0, :],
                                                   op0=MULT, op1=MULT)
                    cs1s.append(cs1); cs2s.append(cs2)

                mre = cs1s[lmap[0]][:, 0, :]
                mim = cs1s[lmap[0]][:, 1, :]
                P2 = None
                for l in range(N_LAYERS):
                    li = lmap[l]
                    if l > 0:
                        sd1 = dapply(P2, cs1s[li])
                        mre, mim = sd1[:, 0, :], sd1[:, 1, :]
                    P1 = gp.tile([128, 2, FCOL], f32, tag="P1")
                    cgate(P1, 0, mre, mim)
                    sd2 = dapply(P1, cs2s[li])
                    P2 = gp.tile([128, 2, FCOL], f32, tag="P2")
                    cgate(P2, 1 + l, sd2[:, 0, :], sd2[:, 1, :])

                sq = sqp.tile([128, 2, FCOL], dt_st, tag="sqt")
                nc.scalar.activation(sq[:], P2[:], ACT.Square)
                outp = op.tile([32, FCOL], f32)
                nc.tensor.matmul(outp[:], sgm[:], sq[:, 0, :],
                                 start=True, stop=False)
                nc.tensor.matmul(outp[:], sgm[:], sq[:, 1, :],
                                 start=False, stop=True)
                ot = sqp.tile([32, FCOL], f32, tag="ot")
                nc.scalar.copy(ot[:], outp[:])
                ta = tpp.tile([128, 4, 32], f32, tag="x")
                for k in range(4):
                    nc.tensor.transpose(ta[:, k, :],
                                        ot[:, 128 * k:128 * (k + 1)], id32[:])
                nc.scalar.copy(od_all[:, t, :],
                               ta[:].rearrange("p a b -> p (a b)"))

            # ---- final store --------------------------------------------
            oflat = out_ap[:].rearrange("(p s) w -> p (s w)", p=128)
            nc.sync.dma_start(oflat, od_all[:].rearrange("p a b -> p (a b)"))

    nc.compile()
    return nc


def _get_nc(bs, u, lmap, dt_state_name=None):
    key = (bs, u, tuple(lmap), dt_state_name or DT_STATE)
    if key not in _NC_CACHE:
        _NC_CACHE[key] = _build_nc(bs, u, lmap, dt_state_name)
    return _NC_CACHE[key]


def _np_dt(name):
    import ml_dtypes
    return {"float32": np.float32, "float32r": np.float32,
            "float16": np.float16, "bfloat16": ml_dtypes.bfloat16}[name]


def kernel(x, weights, scaling):
    from concourse.bass_utils import run_bass_kernel_spmd

    x = np.ascontiguousarray(np.asarray(x, dtype=np.float32))
    B = x.shape[0]
    consts, u, lmap = _host_tensors(weights, scaling,
                                    dt_state=_np_dt(DT_STATE))

    chunk = N_CORES * F
    Bp = ((B + chunk - 1) // chunk) * chunk
    if Bp != B:
        xp = np.zeros((Bp, x.shape[1]), dtype=np.float32)
        xp[:B] = x
        x = xp
    bs = Bp // N_CORES

    nc = _get_nc(bs, u, lmap)
    xs = x.reshape(N_CORES, bs, x.shape[1])
    in_maps = [dict(consts, x=np.ascontiguousarray(xs[i]))
               for i in range(N_CORES)]
    res = run_bass_kernel_spmd(nc, in_maps, core_ids=list(range(N_CORES)))
    out = np.concatenate([r["out"] for r in res.results], axis=0)
    return out[:B]
